# revision 2
# baseline (speedup 1.0000x reference)
"""Trainium2 Bass kernel v3d for nn_KroneckerAddress (topk of Kronecker softmax).

Full inputs: z [64, 384] f32, log_tau [1] f32. Returns (indices [64,32] i32,
weights [64,32] f32) matching the jax reference bit-exactly on this device.

Structure:
- Compact [24,128] layout: partition u = factor*8 + row. ONE joint softmax +
  factor top-32 pass instead of three (DVE time scales with free axis only).
- Softmax bit-matches the reference lowering (max-sub, ACT exp, reduce-add,
  reciprocal, multiply) so every candidate product equals the reference's.
- Replication to 16-partition row groups via one-hot PE matmuls, split
  vals-first so stage-A products start as early as possible.
- Index bookkeeping (pair/triple flat index arrays, wrapped positions via
  scalar_tensor_tensor accumulate, indirect_copy gathers) on GpSimd,
  concurrent with the DVE stage top-ks.
- 164-slot stage candidate set covering {(a+1)(b+1) <= 32}.
- Inputs DMA'd at t~0 from Sync/GpSimd; constants generated on device.

Sharding: pure data parallel, 8 rows per core across 8 cores.
"""
import sys

sys.path.insert(0, '/opt/trn_rl_repo')

import json
import os

import numpy as np

import concourse.bass as bass
import concourse.mybir as mybir
from concourse.tile import TileContext
from concourse.bass_utils import run_bass_kernel_spmd
import concourse.bass2jax as _b2j

f32 = mybir.dt.float32
u16 = mybir.dt.uint16
u32 = mybir.dt.uint32

B, U, DP, K = 64, 3, 128, 32
NCORES = 8
RPC = B // NCORES          # rows per core
NU = U * RPC               # compact layout: partition u = factor*8 + row
NP = 128
# stage candidate rectangles (factor-a rank range, b count); covers
# {(a,b): (a+1)(b+1) <= 32}. 64+60+24+16 = 164 slots.
SEGS = [(0, 2, 32), (2, 8, 10), (8, 16, 3), (16, 32, 1)]
NSLOT = sum((a1 - a0) * bc for a0, a1, bc in SEGS)
RW = [115, 146, 156, 164]   # per-round rank-bound prefix widths


# ---------------------------------------------------------------------------
# This container's walrus build rejects instructions with >1 sync wait.
# Split multi-wait instructions into single-wait Drains on the same engine
# placed immediately before them (per-engine program order => equivalent).
def _split_multiwaits(bir_bytes: bytes) -> bytes:
    d = json.loads(bir_bytes)
    ctr = 0
    changed = False
    for fn in d.get('functions', []):
        for bb in fn.get('blocks', []):
            new_insts = []
            for inst in bb.get('instructions', []):
                si = inst.get('sync_info')
                ow = (si or {}).get('on_wait') or []
                eng = inst.get('engine', 'Unassigned')
                if len(ow) > 1 and eng != 'Unassigned':
                    for w in ow[:-1]:
                        ctr += 1
                        new_insts.append({
                            'debug': inst.get('debug', 0),
                            'engine': eng,
                            'ins': [],
                            'outs': [],
                            'name': f"WS-{ctr}-{inst['name']}",
                            'opcode': 'Drain',
                            'sync_info': {'on_update': [], 'on_wait': [w]},
                        })
                    si['on_wait'] = ow[-1:]
                    changed = True
                new_insts.append(inst)
            bb['instructions'] = new_insts
    return json.dumps(d).encode() if changed else bir_bytes


_orig_compile = _b2j.compile_bir_kernel


def _patched_compile(ant_bir_str, *args, **kwargs):
    return _orig_compile(_split_multiwaits(ant_bir_str), *args, **kwargs)


if _b2j.compile_bir_kernel.__name__ != '_patched_compile':
    _b2j.compile_bir_kernel = _patched_compile
# ---------------------------------------------------------------------------


def _bcast(t_ap, col0, pat):
    """AP over tile `t_ap`'s partitions starting at free column col0 with
    custom free [step, count] dims (step 0 = broadcast)."""
    base = t_ap[:, col0:col0 + 1]
    return bass.AP(tensor=base.tensor, offset=base.offset,
                   ap=[base.ap[0]] + [list(p) for p in pat])


def _pstride(t_ap, pstep, pcount, width):
    """AP reading `pcount` partitions with stride `pstep`, `width` cols.
    Partition stride is in flattened elements: pstep * per-partition row len."""
    row = t_ap.ap[0][0]
    return bass.AP(tensor=t_ap.tensor, offset=t_ap.offset,
                   ap=[[pstep * row, pcount], [1, width]])


def _stage_products(nc, eng, out_s, in_a, in_b, op):
    """out_s[:, slot] = in_a[:, a] (op) in_b[:, b] over the SEGS layout."""
    col = 0
    for a0, a1, bcnt in SEGS:
        na = a1 - a0
        n = na * bcnt
        eng.tensor_tensor(
            out=out_s[:, col:col + n].rearrange("p (a b) -> p a b", a=na),
            in0=_bcast(in_a, a0, [[1, na], [0, bcnt]]),
            in1=_bcast(in_b, 0, [[0, na], [1, bcnt]]),
            op=op,
        )
        col += n


def _topk32(nc, work, width, vals, pos, sentinel, widths=None):
    """Top-32 of work[:, :width] per partition -> vals [.,32] f32 descending,
    pos [.,32] u16 positions. Mutates work (found values -> sentinel).
    widths: optional per-round prefix widths (round r's top-8 provably lies
    within the first widths[r] slots; each must cover the previous)."""
    if widths is None:
        widths = [width] * 4
    for r in range(4):
        w = widths[r]
        v8 = vals[:, r * 8:(r + 1) * 8]
        nc.vector.max(out=v8, in_=work[:, :w])
        nc.vector.max_index(out=pos[:, r * 8:(r + 1) * 8], in_max=v8,
                            in_values=work[:, :w])
        if r < 3:
            nc.vector.match_replace(out=work[:, :w], in_to_replace=v8,
                                    in_values=work[:, :w],
                                    imm_value=sentinel)


def _wrap(nc, pool, pos, mdiag, name):
    """Wrapped gather positions: wpos[p, s] = pos[p, s*16 + p%16] via
    per-half multiply-accumulate on gpsimd (u16, sums < 256, exact)."""
    OP = mybir.AluOpType
    scr = pool.tile([NP, K], u16, name=f"wscr{name}")
    wpos = pool.tile([NP, 2], u16, name=f"wpos{name}")
    nc.vector.tensor_tensor(
        out=scr[:, :].rearrange("p (s w) -> p s w", s=2),
        in0=pos[:, :].rearrange("p (s w) -> p s w", s=2),
        in1=_bcast(mdiag, 0, [[0, 2], [1, 16]]),
        op=OP.mult)
    with nc.allow_low_precision(reason="u16 pos sums < 256, exact"):
        nc.vector.tensor_reduce(
            out=wpos[:, :],
            in_=scr[:, :].rearrange("p (s w) -> p s w", s=2),
            axis=mybir.AxisListType.X, op=OP.add)
    return wpos


STOP_AFTER = int(os.environ.get("KSTOP", "99"))


def _build_module():
    nc = bass.Bass()
    z_d = nc.dram_tensor("z", [RPC, U * DP], f32, kind="ExternalInput")
    lt_d = nc.dram_tensor("log_tau", [1, 1], f32, kind="ExternalInput")
    w_d = nc.dram_tensor("w_out", [RPC, K], f32, kind="ExternalOutput")
    i_d = nc.dram_tensor("idx_out", [RPC, K], f32, kind="ExternalOutput")

    AF = mybir.ActivationFunctionType
    OP = mybir.AluOpType

    with TileContext(nc) as tc:
        with tc.tile_pool(name="p", bufs=1) as pool, \
             tc.tile_pool(name="pp", bufs=1, space="PSUM") as ppool:
            # ---- iotas first (unblock DVE-side mdiag setup), then input
            # DMAs (z on sync, lt on gpsimd), then bigger consts ----
            zt = pool.tile([NU, DP], f32)
            z_ap = z_d[:, :]
            # u = f*8 + r -> dram offset f*DP + r*U*DP; split factors {0,1}
            # (sync) / factor 2 (scalar) for lower input latency
            z01 = bass.AP(tensor=z_ap.tensor, offset=z_ap.offset,
                          ap=[[DP, 2], [U * DP, RPC], [1, DP]])
            z2 = bass.AP(tensor=z_ap.tensor, offset=z_ap.offset + 2 * DP,
                         ap=[[U * DP, RPC], [1, DP]])
            nc.sync.dma_start(zt[0:2 * RPC, :], z01)
            nc.scalar.dma_start(zt[2 * RPC:NU, :], z2)
            lt = pool.tile([NU, 1], f32)
            lt_ap = lt_d[:, :]
            lt_src = bass.AP(tensor=lt_ap.tensor, offset=lt_ap.offset,
                             ap=[[0, NU], [1, 1]])
            nc.gpsimd.dma_start(lt[:, :], lt_src)
            pid = pool.tile([NP, 1], u32)
            nc.gpsimd.iota(pid[:, :], pattern=[[0, 1]], base=0,
                           channel_multiplier=1)
            iw = pool.tile([NP, 16], u32)
            nc.gpsimd.iota(iw[:, :], pattern=[[1, 16]], base=0,
                           channel_multiplier=0)

            # wrap-diagonal mdiag[p, w] = (w == p % 16); DVE runs these
            # while waiting for z
            pmod = pool.tile([NP, 1], u32)
            nc.vector.tensor_scalar(out=pmod[:, :], in0=pid[:, :],
                                    scalar1=15, scalar2=None,
                                    op0=OP.bitwise_and)
            pmodf = pool.tile([NP, 1], f32)
            nc.vector.tensor_copy(pmodf[:, :], pmod[:, :])
            iwf = pool.tile([NP, 16], f32)
            nc.vector.tensor_copy(iwf[:, :], iw[:, :])
            mdiag = pool.tile([NP, 16], u16)
            nc.vector.tensor_scalar(out=mdiag[:, :], in0=iwf[:, :],
                                    scalar1=pmodf[:, 0:1], scalar2=None,
                                    op0=OP.is_equal)

            # one-hot replication matrices M[f][u, d] = (u == f*8 + d//16)
            ones24 = pool.tile([NU, NP], f32)
            nc.gpsimd.memset(ones24[:, :], 1.0)
            M = []
            Mi = []
            for f in range(U):
                mf = pool.tile([NU, NP], f32, name=f"M{f}")
                nc.gpsimd.affine_select(
                    out=mf[:, :].rearrange("p (a b) -> p a b", a=RPC),
                    in_=ones24[:, :].rearrange("p (a b) -> p a b", a=RPC),
                    pattern=[[-1, RPC], [0, 16]],
                    base=-8 * f, channel_multiplier=1,
                    compare_op=OP.is_equal, fill=0.0)
                M.append(mf)
            for f, sc in ((0, 16384.0), (1, 128.0)):
                mfi = pool.tile([NU, NP], f32, name=f"Mi{f}")
                nc.gpsimd.tensor_scalar(out=mfi[:, :], in0=M[f][:, :],
                                        scalar1=sc, scalar2=None,
                                        op0=OP.mult)
                Mi.append(mfi)
            Mi.append(M[2])

            # ---- softmax, bit-matching the reference lowering ----
            rtau = pool.tile([NU, 1], f32)
            nc.scalar.activation(out=rtau[:, :], in_=lt[:, :], func=AF.Exp,
                                 bias=0.0, scale=-1.0)
            nm = pool.tile([NU, 1], f32)
            nc.vector.tensor_reduce(out=nm[:, :], in_=zt[:, :],
                                    axis=mybir.AxisListType.X,
                                    op=OP.max, negate=True)
            # rhs for replication matmuls: [vals | rowsum-recip | fidx*128]
            rhs = pool.tile([NU, 2 * K + 1], f32)
            et = pool.tile([NU, DP], f32)
            s = pool.tile([NU, 1], f32)
            nc.scalar.activation(out=et[:, :], in_=zt[:, :], func=AF.Exp,
                                 bias=nm[:, 0:1], scale=rtau[:, 0:1],
                                 accum_out=s[:, :])

            if STOP_AFTER == 1:
                nc.gpsimd.dma_start(w_d[:, :], _pstride(et, 1, RPC, K))
                nc.gpsimd.dma_start(i_d[:, :], _pstride(rhs, 1, RPC, K))
                return nc

            # ---- joint factor top-32 on raw exp values (normalization is
            # a uniform positive per-unit scale: same order, verified
            # tie-free), then normalize just the 32 selected values ----
            fidx = pool.tile([NU, K], u16)
            for r in range(4):
                v8 = rhs[:, r * 8:(r + 1) * 8]
                nc.vector.max(out=v8, in_=et[:, :])
                if r == 0:
                    nc.vector.reciprocal(rhs[:, K:K + 1], s[:, :])
                nc.vector.max_index(out=fidx[:, r * 8:(r + 1) * 8],
                                    in_max=v8, in_values=et[:, :])
                if r < 3:
                    nc.vector.match_replace(out=et[:, :], in_to_replace=v8,
                                            in_values=et[:, :],
                                            imm_value=-1.0)
            nc.vector.tensor_copy(rhs[:, K + 1:2 * K + 1], fidx[:, :])

            # ---- replicate to 16-partition row groups via PE, vals first
            # so DVE stage-A products can start ASAP ----
            pv, pf = [], []
            for f in range(U):
                t = ppool.tile([NP, K + 1], f32, name=f"pv{f}")
                nc.tensor.matmul(t[:, :], M[f][:, :], rhs[:, 0:K + 1],
                                 start=True, stop=True)
                pv.append(t)
            for f in range(U):
                t = ppool.tile([NP, K], f32, name=f"pf{f}")
                nc.tensor.matmul(t[:, :], Mi[f][:, :],
                                 rhs[:, K + 1:2 * K + 1],
                                 start=True, stop=True)
                pf.append(t)
            # normalize replicated vals into SBUF (scalar input from PSUM is
            # exempt from the one-PSUM-input rule); fidx halves via ACT
            vn = []
            for f in range(U):
                t = pool.tile([NP, K], f32, name=f"vn{f}")
                nc.vector.tensor_scalar(out=t[:, :], in0=pv[f][:, 0:K],
                                        scalar1=pv[f][:, K:K + 1],
                                        scalar2=None, op0=OP.mult)
                vn.append(t)
            v0sb = vn[0]
            fsb = []
            for f in range(U):
                ff = pool.tile([NP, K], f32, name=f"fsb{f}")
                nc.scalar.activation(out=ff[:, :], in_=pf[f][:, :],
                                     func=AF.Copy, bias=0.0, scale=1.0)
                fsb.append(ff)

            if STOP_AFTER == 2:
                nc.gpsimd.dma_start(w_d[:, :], _pstride(v0sb, 16, RPC, K))
                nc.gpsimd.dma_start(i_d[:, :], _pstride(fsb[0], 16, RPC, K))
                return nc

            # ---- stage A: pair products (DVE) + pair flat idx (gpsimd) ----
            sA = pool.tile([NP, NSLOT], f32)
            _stage_products(nc, nc.vector, sA, vn[0], vn[1], OP.mult)
            nA = pool.tile([NP, NSLOT], f32)
            _stage_products(nc, nc.gpsimd, nA, fsb[0], fsb[1], OP.add)

            vA = pool.tile([NP, K], f32)
            posA = pool.tile([NP, K], u16)
            _topk32(nc, sA, NSLOT, vA, posA, -1.0, widths=RW)

            if STOP_AFTER == 3:
                nc.gpsimd.dma_start(w_d[:, :], _pstride(vA, 16, RPC, K))
                nc.gpsimd.dma_start(i_d[:, :], _pstride(nA, 16, RPC, K))
                return nc

            # ---- gather pair flat indices at posA (gpsimd) ----
            wposA = _wrap(nc, pool, posA, mdiag, "A")
            gA = pool.tile([NP, K], f32)
            nc.gpsimd.indirect_copy(out=gA[:, :], data=nA[:, :],
                                    idxs=wposA[:, :],
                                    i_know_ap_gather_is_preferred=True)

            # ---- stage B: triple products + flat indices ----
            sB = pool.tile([NP, NSLOT], f32)
            _stage_products(nc, nc.vector, sB, vA, vn[2], OP.mult)
            nB = pool.tile([NP, NSLOT], f32)
            _stage_products(nc, nc.gpsimd, nB, gA, fsb[2], OP.add)

            vB = pool.tile([NP, K], f32)
            posB = pool.tile([NP, K], u16)
            _topk32(nc, sB, NSLOT, vB, posB, -1.0, widths=RW)

            if STOP_AFTER == 4:
                nc.gpsimd.dma_start(w_d[:, :], _pstride(vB, 16, RPC, K))
                nc.gpsimd.dma_start(i_d[:, :], _pstride(nB, 16, RPC, K))
                return nc

            nc.sync.dma_start(w_d[:, :], _pstride(vB, 16, RPC, K))
            wposB = _wrap(nc, pool, posB, mdiag, "B")
            gB = pool.tile([NP, K], f32)
            nc.gpsimd.indirect_copy(out=gB[:, :], data=nB[:, :],
                                    idxs=wposB[:, :],
                                    i_know_ap_gather_is_preferred=True)
            nc.sync.dma_start(i_d[:, :], _pstride(gB, 16, RPC, K))
    return nc


LAST_RESULTS = None


def kernel(z, log_tau, _trace=False):
    z = np.ascontiguousarray(np.asarray(z, dtype=np.float32))
    log_tau = np.asarray(log_tau, dtype=np.float32).reshape(1, 1)
    assert z.shape == (B, U * DP), z.shape

    nc = _build_module()
    in_maps = []
    for c in range(NCORES):
        in_maps.append({
            "z": z[c * RPC:(c + 1) * RPC],
            "log_tau": log_tau,
        })
    global LAST_RESULTS
    kw = {}
    if _trace:
        kw = dict(trace=True, trace_cores=[0])
    res = run_bass_kernel_spmd(nc, in_maps, core_ids=list(range(NCORES)), **kw)
    LAST_RESULTS = res
    indices = np.concatenate([r["idx_out"] for r in res.results], axis=0)
    weights = np.concatenate([r["w_out"] for r in res.results], axis=0)
    return indices.astype(np.int32), weights.astype(np.float32)


if __name__ == "__main__":
    z = np.load('/tmp/z.npy')
    lt = np.load('/tmp/logtau.npy')
    ind, w = kernel(z, lt)
    print(ind[:2], w[:2])



# revision 3
# speedup vs baseline: 1.1585x; 1.1585x over previous
"""Trainium2 Bass kernel v3m for nn_KroneckerAddress (topk of Kronecker softmax).

Full inputs: z [64, 384] f32, log_tau [1] f32. Returns (indices [64,32] i32,
weights [64,32] f32) matching the jax reference bit-exactly on this device.

Structure:
- Compact [24,128] layout: partition u = factor*8 + row. ONE joint softmax +
  factor top-32 pass instead of three (DVE time scales with free axis only).
- Softmax bit-matches the reference lowering (max-sub, ACT exp, reduce-add,
  reciprocal, multiply) so every candidate product equals the reference's.
- Replication to 16-partition row groups via one-hot PE matmuls, split
  vals-first so stage-A products start as early as possible.
- Index bookkeeping (pair/triple flat index arrays, wrapped positions via
  scalar_tensor_tensor accumulate, indirect_copy gathers) on GpSimd,
  concurrent with the DVE stage top-ks.
- 164-slot stage candidate set covering {(a+1)(b+1) <= 32}.
- Inputs DMA'd at t~0 from Sync/GpSimd; constants generated on device.

Sharding: pure data parallel, 8 rows per core across 8 cores.
"""
import sys

sys.path.insert(0, '/opt/trn_rl_repo')

import json
import os

import numpy as np

import concourse.bass as bass
import concourse.mybir as mybir
from concourse.tile import TileContext
from concourse.bass_utils import run_bass_kernel_spmd
import concourse.bass2jax as _b2j

f32 = mybir.dt.float32
u16 = mybir.dt.uint16
u32 = mybir.dt.uint32

B, U, DP, K = 64, 3, 128, 32
NCORES = 8
RPC = B // NCORES          # rows per core
NU = U * RPC               # compact layout: partition u = factor*8 + row
NP = 128
# stage candidate rectangles (factor-a rank range, b count); covers
# {(a,b): (a+1)(b+1) <= 32}. 64+60+24+16 = 164 slots.
SEGS = [(0, 2, 32), (2, 8, 10), (8, 16, 3), (16, 32, 1)]
NSLOT = sum((a1 - a0) * bc for a0, a1, bc in SEGS)
RW = [115, 146, 156, 164]   # per-round rank-bound prefix widths


# ---------------------------------------------------------------------------
# This container's walrus build rejects instructions with >1 sync wait.
# Split multi-wait instructions into single-wait Drains on the same engine
# placed immediately before them (per-engine program order => equivalent).
def _split_multiwaits(bir_bytes: bytes) -> bytes:
    d = json.loads(bir_bytes)
    ctr = 0
    changed = False
    for fn in d.get('functions', []):
        for bb in fn.get('blocks', []):
            new_insts = []
            for inst in bb.get('instructions', []):
                si = inst.get('sync_info')
                ow = (si or {}).get('on_wait') or []
                eng = inst.get('engine', 'Unassigned')
                if len(ow) > 1 and eng != 'Unassigned':
                    for w in ow[:-1]:
                        ctr += 1
                        new_insts.append({
                            'debug': inst.get('debug', 0),
                            'engine': eng,
                            'ins': [],
                            'outs': [],
                            'name': f"WS-{ctr}-{inst['name']}",
                            'opcode': 'Drain',
                            'sync_info': {'on_update': [], 'on_wait': [w]},
                        })
                    si['on_wait'] = ow[-1:]
                    changed = True
                new_insts.append(inst)
            bb['instructions'] = new_insts
    return json.dumps(d).encode() if changed else bir_bytes


_orig_compile = _b2j.compile_bir_kernel


def _patched_compile(ant_bir_str, *args, **kwargs):
    return _orig_compile(_split_multiwaits(ant_bir_str), *args, **kwargs)


if _b2j.compile_bir_kernel.__name__ != '_patched_compile':
    _b2j.compile_bir_kernel = _patched_compile
# ---------------------------------------------------------------------------


def _bcast(t_ap, col0, pat):
    """AP over tile `t_ap`'s partitions starting at free column col0 with
    custom free [step, count] dims (step 0 = broadcast)."""
    base = t_ap[:, col0:col0 + 1]
    return bass.AP(tensor=base.tensor, offset=base.offset,
                   ap=[base.ap[0]] + [list(p) for p in pat])


def _pstride(t_ap, pstep, pcount, width):
    """AP reading `pcount` partitions with stride `pstep`, `width` cols.
    Partition stride is in flattened elements: pstep * per-partition row len."""
    row = t_ap.ap[0][0]
    return bass.AP(tensor=t_ap.tensor, offset=t_ap.offset,
                   ap=[[pstep * row, pcount], [1, width]])


def _stage_products(nc, eng, out_s, in_a, in_b, op):
    """out_s[:, slot] = in_a[:, a] (op) in_b[:, b] over the SEGS layout."""
    col = 0
    for a0, a1, bcnt in SEGS:
        na = a1 - a0
        n = na * bcnt
        eng.tensor_tensor(
            out=out_s[:, col:col + n].rearrange("p (a b) -> p a b", a=na),
            in0=_bcast(in_a, a0, [[1, na], [0, bcnt]]),
            in1=_bcast(in_b, 0, [[0, na], [1, bcnt]]),
            op=op,
        )
        col += n


def _topk32(nc, work, width, vals, pos, sentinel, widths=None):
    """Top-32 of work[:, :width] per partition -> vals [.,32] f32 descending,
    pos [.,32] u16 positions. Mutates work (found values -> sentinel).
    widths: optional per-round prefix widths (round r's top-8 provably lies
    within the first widths[r] slots; each must cover the previous)."""
    if widths is None:
        widths = [width] * 4
    for r in range(4):
        w = widths[r]
        v8 = vals[:, r * 8:(r + 1) * 8]
        nc.vector.max(out=v8, in_=work[:, :w])
        nc.vector.max_index(out=pos[:, r * 8:(r + 1) * 8], in_max=v8,
                            in_values=work[:, :w])
        if r < 3:
            nc.vector.match_replace(out=work[:, :w], in_to_replace=v8,
                                    in_values=work[:, :w],
                                    imm_value=sentinel)


def _wrap(nc, pool, pos, mdiag, name):
    """Wrapped gather positions: wpos[p, s] = pos[p, s*16 + p%16] via
    per-half multiply-accumulate on gpsimd (u16, sums < 256, exact)."""
    OP = mybir.AluOpType
    scr = pool.tile([NP, K], u16, name=f"wscr{name}")
    wpos = pool.tile([NP, 2], u16, name=f"wpos{name}")
    nc.vector.tensor_tensor(
        out=scr[:, :].rearrange("p (s w) -> p s w", s=2),
        in0=pos[:, :].rearrange("p (s w) -> p s w", s=2),
        in1=_bcast(mdiag, 0, [[0, 2], [1, 16]]),
        op=OP.mult)
    with nc.allow_low_precision(reason="u16 pos sums < 256, exact"):
        nc.vector.tensor_reduce(
            out=wpos[:, :],
            in_=scr[:, :].rearrange("p (s w) -> p s w", s=2),
            axis=mybir.AxisListType.X, op=OP.add)
    return wpos


STOP_AFTER = int(os.environ.get("KSTOP", "99"))


def _build_module():
    nc = bass.Bass()
    z_d = nc.dram_tensor("z", [RPC, U * DP], f32, kind="ExternalInput")
    lt_d = nc.dram_tensor("log_tau", [1, 1], f32, kind="ExternalInput")
    w_d = nc.dram_tensor("w_out", [RPC, K], f32, kind="ExternalOutput")
    i_d = nc.dram_tensor("idx_out", [RPC, K], f32, kind="ExternalOutput")

    AF = mybir.ActivationFunctionType
    OP = mybir.AluOpType

    with TileContext(nc) as tc:
        with tc.tile_pool(name="p", bufs=1) as pool, \
             tc.tile_pool(name="pp", bufs=1, space="PSUM") as ppool:
            # ---- iotas first (unblock DVE-side mdiag setup), then input
            # DMAs (z on sync, lt on gpsimd), then bigger consts ----
            zt = pool.tile([NU, DP], f32)
            z_ap = z_d[:, :]
            # u = f*8 + r -> dram offset f*DP + r*U*DP; split factors {0,1}
            # (sync) / factor 2 (scalar) for lower input latency
            z01 = bass.AP(tensor=z_ap.tensor, offset=z_ap.offset,
                          ap=[[DP, 2], [U * DP, RPC], [1, DP]])
            z2 = bass.AP(tensor=z_ap.tensor, offset=z_ap.offset + 2 * DP,
                         ap=[[U * DP, RPC], [1, DP]])
            nc.sync.dma_start(zt[0:2 * RPC, :], z01)
            nc.scalar.dma_start(zt[2 * RPC:NU, :], z2)
            lt = pool.tile([NU, 1], f32)
            lt_ap = lt_d[:, :]
            lt_src = bass.AP(tensor=lt_ap.tensor, offset=lt_ap.offset,
                             ap=[[0, NU], [1, 1]])
            nc.gpsimd.dma_start(lt[:, :], lt_src)
            pid = pool.tile([NP, 1], u32)
            nc.gpsimd.iota(pid[:, :], pattern=[[0, 1]], base=0,
                           channel_multiplier=1)
            iw = pool.tile([NP, 16], u32)
            nc.gpsimd.iota(iw[:, :], pattern=[[1, 16]], base=0,
                           channel_multiplier=0)

            # wrap-diagonal mdiag[p, w] = (w == p % 16); DVE runs these
            # while waiting for z
            pmod = pool.tile([NP, 1], u32)
            nc.vector.tensor_scalar(out=pmod[:, :], in0=pid[:, :],
                                    scalar1=15, scalar2=None,
                                    op0=OP.bitwise_and)
            pmodf = pool.tile([NP, 1], f32)
            nc.vector.tensor_copy(pmodf[:, :], pmod[:, :])
            iwf = pool.tile([NP, 16], f32)
            nc.vector.tensor_copy(iwf[:, :], iw[:, :])
            mdiag = pool.tile([NP, 16], u16)
            nc.vector.tensor_scalar(out=mdiag[:, :], in0=iwf[:, :],
                                    scalar1=pmodf[:, 0:1], scalar2=None,
                                    op0=OP.is_equal)
            # warm-up gather in the idle const window: if the Pool engine
            # caches the indirect-copy ucode, the two real gathers on the
            # index tail launch without the cold-start penalty
            wrmi = pool.tile([NP, 2], u16)
            nc.gpsimd.memset(wrmi[:, :], 0)
            wrmo = pool.tile([NP, 8], u16)
            nc.gpsimd.indirect_copy(out=wrmo[:, :], data=mdiag[:, :],
                                    idxs=wrmi[:, :],
                                    i_know_ap_gather_is_preferred=True)

            # one-hot replication matrices M[f][u, d] = (u == f*8 + d//16)
            ones24 = pool.tile([NU, NP], f32)
            nc.gpsimd.memset(ones24[:, :], 1.0)
            M = []
            Mi = []
            for f in range(U):
                mf = pool.tile([NU, NP], f32, name=f"M{f}")
                nc.gpsimd.affine_select(
                    out=mf[:, :].rearrange("p (a b) -> p a b", a=RPC),
                    in_=ones24[:, :].rearrange("p (a b) -> p a b", a=RPC),
                    pattern=[[-1, RPC], [0, 16]],
                    base=-8 * f, channel_multiplier=1,
                    compare_op=OP.is_equal, fill=0.0)
                M.append(mf)
            for f, sc in ((0, 16384.0), (1, 128.0)):
                mfi = pool.tile([NU, NP], f32, name=f"Mi{f}")
                nc.gpsimd.tensor_scalar(out=mfi[:, :], in0=M[f][:, :],
                                        scalar1=sc, scalar2=None,
                                        op0=OP.mult)
                Mi.append(mfi)
            Mi.append(M[2])

            # ---- softmax, bit-matching the reference lowering ----
            rtau = pool.tile([NU, 1], f32)
            nc.scalar.activation(out=rtau[:, :], in_=lt[:, :], func=AF.Exp,
                                 bias=0.0, scale=-1.0)
            nm = pool.tile([NU, 1], f32)
            nc.vector.tensor_reduce(out=nm[:, :], in_=zt[:, :],
                                    axis=mybir.AxisListType.X,
                                    op=OP.max, negate=True)
            # rhs for replication matmuls: [vals | rowsum-recip | fidx*128]
            rhs = pool.tile([NU, 2 * K + 1], f32)
            et = pool.tile([NU, DP], f32)
            s = pool.tile([NU, 1], f32)
            nc.scalar.activation(out=et[:, :], in_=zt[:, :], func=AF.Exp,
                                 bias=nm[:, 0:1], scale=rtau[:, 0:1],
                                 accum_out=s[:, :])

            if STOP_AFTER == 1:
                nc.gpsimd.dma_start(w_d[:, :], _pstride(et, 1, RPC, K))
                nc.gpsimd.dma_start(i_d[:, :], _pstride(rhs, 1, RPC, K))
                return nc

            # ---- joint factor top-32 on raw exp values (normalization is
            # a uniform positive per-unit scale: same order, verified
            # tie-free), then normalize just the 32 selected values ----
            fidx = pool.tile([NU, K], u16)
            for r in range(4):
                v8 = rhs[:, r * 8:(r + 1) * 8]
                nc.vector.max(out=v8, in_=et[:, :])
                if r == 0:
                    nc.vector.reciprocal(rhs[:, K:K + 1], s[:, :])
                nc.vector.max_index(out=fidx[:, r * 8:(r + 1) * 8],
                                    in_max=v8, in_values=et[:, :])
                if r < 3:
                    nc.vector.match_replace(out=et[:, :], in_to_replace=v8,
                                            in_values=et[:, :],
                                            imm_value=-1.0)
            nc.vector.tensor_copy(rhs[:, K + 1:2 * K + 1], fidx[:, :])

            # ---- replicate to 16-partition row groups via PE, vals first
            # so DVE stage-A products can start ASAP ----
            pv, pf = [], []
            for f in range(U):
                t = ppool.tile([NP, K + 1], f32, name=f"pv{f}")
                nc.tensor.matmul(t[:, :], M[f][:, :], rhs[:, 0:K + 1],
                                 start=True, stop=True)
                pv.append(t)
            for f in range(U):
                t = ppool.tile([NP, K], f32, name=f"pf{f}")
                nc.tensor.matmul(t[:, :], Mi[f][:, :],
                                 rhs[:, K + 1:2 * K + 1],
                                 start=True, stop=True)
                pf.append(t)
            # normalize replicated vals into SBUF (scalar input from PSUM is
            # exempt from the one-PSUM-input rule); fidx halves via ACT
            vn = []
            for f in range(U):
                t = pool.tile([NP, K], f32, name=f"vn{f}")
                nc.vector.tensor_scalar(out=t[:, :], in0=pv[f][:, 0:K],
                                        scalar1=pv[f][:, K:K + 1],
                                        scalar2=None, op0=OP.mult)
                vn.append(t)
            v0sb = vn[0]
            fsb = []
            for f in range(U):
                ff = pool.tile([NP, K], f32, name=f"fsb{f}")
                nc.scalar.activation(out=ff[:, :], in_=pf[f][:, :],
                                     func=AF.Copy, bias=0.0, scale=1.0)
                fsb.append(ff)

            if STOP_AFTER == 2:
                nc.gpsimd.dma_start(w_d[:, :], _pstride(v0sb, 16, RPC, K))
                nc.gpsimd.dma_start(i_d[:, :], _pstride(fsb[0], 16, RPC, K))
                return nc

            # ---- stage A: pair products (DVE) + pair flat idx (gpsimd) ----
            sA = pool.tile([NP, NSLOT], f32)
            _stage_products(nc, nc.vector, sA, vn[0], vn[1], OP.mult)
            nA = pool.tile([NP, NSLOT], f32)
            _stage_products(nc, nc.gpsimd, nA, fsb[0], fsb[1], OP.add)

            vA = pool.tile([NP, K], f32)
            posA = pool.tile([NP, K], u16)
            _topk32(nc, sA, NSLOT, vA, posA, -1.0, widths=RW)

            if STOP_AFTER == 3:
                nc.gpsimd.dma_start(w_d[:, :], _pstride(vA, 16, RPC, K))
                nc.gpsimd.dma_start(i_d[:, :], _pstride(nA, 16, RPC, K))
                return nc

            # ---- gather pair flat indices at posA (gpsimd) ----
            wposA = _wrap(nc, pool, posA, mdiag, "A")
            gA = pool.tile([NP, K], f32)
            nc.gpsimd.indirect_copy(out=gA[:, :], data=nA[:, :],
                                    idxs=wposA[:, :],
                                    i_know_ap_gather_is_preferred=True)

            # ---- stage B: triple products + flat indices ----
            sB = pool.tile([NP, NSLOT], f32)
            _stage_products(nc, nc.vector, sB, vA, vn[2], OP.mult)
            nB = pool.tile([NP, NSLOT], f32)
            _stage_products(nc, nc.gpsimd, nB, gA, fsb[2], OP.add)

            vB = pool.tile([NP, K], f32)
            posB = pool.tile([NP, K], u16)
            _topk32(nc, sB, NSLOT, vB, posB, -1.0, widths=RW)

            if STOP_AFTER == 4:
                nc.gpsimd.dma_start(w_d[:, :], _pstride(vB, 16, RPC, K))
                nc.gpsimd.dma_start(i_d[:, :], _pstride(nB, 16, RPC, K))
                return nc

            nc.sync.dma_start(w_d[:, :], _pstride(vB, 16, RPC, K))
            wposB = _wrap(nc, pool, posB, mdiag, "B")
            gB = pool.tile([NP, K], f32)
            nc.gpsimd.indirect_copy(out=gB[:, :], data=nB[:, :],
                                    idxs=wposB[:, :],
                                    i_know_ap_gather_is_preferred=True)
            nc.sync.dma_start(i_d[:, :], _pstride(gB, 16, RPC, K))
    return nc


LAST_RESULTS = None


def kernel(z, log_tau, _trace=False):
    z = np.ascontiguousarray(np.asarray(z, dtype=np.float32))
    log_tau = np.asarray(log_tau, dtype=np.float32).reshape(1, 1)
    assert z.shape == (B, U * DP), z.shape

    nc = _build_module()
    in_maps = []
    for c in range(NCORES):
        in_maps.append({
            "z": z[c * RPC:(c + 1) * RPC],
            "log_tau": log_tau,
        })
    global LAST_RESULTS
    kw = {}
    if _trace:
        kw = dict(trace=True, trace_cores=[0])
    res = run_bass_kernel_spmd(nc, in_maps, core_ids=list(range(NCORES)), **kw)
    LAST_RESULTS = res
    indices = np.concatenate([r["idx_out"] for r in res.results], axis=0)
    weights = np.concatenate([r["w_out"] for r in res.results], axis=0)
    return indices.astype(np.int32), weights.astype(np.float32)


if __name__ == "__main__":
    z = np.load('/tmp/z.npy')
    lt = np.load('/tmp/logtau.npy')
    ind, w = kernel(z, lt)
    print(ind[:2], w[:2])



# revision 4
# speedup vs baseline: 1.1616x; 1.0027x over previous
"""Trainium2 Bass kernel v3d for nn_KroneckerAddress (topk of Kronecker softmax).

Full inputs: z [64, 384] f32, log_tau [1] f32. Returns (indices [64,32] i32,
weights [64,32] f32) matching the jax reference bit-exactly on this device.

Structure:
- Compact [24,128] layout: partition u = factor*8 + row. ONE joint softmax +
  factor top-32 pass instead of three (DVE time scales with free axis only).
- Softmax bit-matches the reference lowering (max-sub, ACT exp, reduce-add,
  reciprocal, multiply) so every candidate product equals the reference's.
- Replication to 16-partition row groups via one-hot PE matmuls, split
  vals-first so stage-A products start as early as possible.
- Index bookkeeping (pair/triple flat index arrays, wrapped positions via
  scalar_tensor_tensor accumulate, indirect_copy gathers) on GpSimd,
  concurrent with the DVE stage top-ks.
- 164-slot stage candidate set covering {(a+1)(b+1) <= 32}.
- Inputs DMA'd at t~0 from Sync/GpSimd; constants generated on device.

Sharding: pure data parallel, 8 rows per core across 8 cores.
"""
import sys

sys.path.insert(0, '/opt/trn_rl_repo')

import json
import os

import numpy as np

import concourse.bass as bass
import concourse.mybir as mybir
from concourse.tile import TileContext
from concourse.bass_utils import run_bass_kernel_spmd
import concourse.bass2jax as _b2j

f32 = mybir.dt.float32
u16 = mybir.dt.uint16
u32 = mybir.dt.uint32

B, U, DP, K = 64, 3, 128, 32
NCORES = 8
RPC = B // NCORES          # rows per core
NU = U * RPC               # compact layout: partition u = factor*8 + row
NP = 128
# stage candidate rectangles (factor-a rank range, b count); covers
# {(a,b): (a+1)(b+1) <= 32}. 64+60+24+16 = 164 slots.
SEGS = [(0, 2, 32), (2, 8, 10), (8, 16, 3), (16, 32, 1)]
NSLOT = sum((a1 - a0) * bc for a0, a1, bc in SEGS)
RW = [115, 146, 156, 164]   # per-round rank-bound prefix widths


# ---------------------------------------------------------------------------
# This container's walrus build rejects instructions with >1 sync wait.
# Split multi-wait instructions into single-wait Drains on the same engine
# placed immediately before them (per-engine program order => equivalent).
def _split_multiwaits(bir_bytes: bytes) -> bytes:
    d = json.loads(bir_bytes)
    ctr = 0
    changed = False
    for fn in d.get('functions', []):
        for bb in fn.get('blocks', []):
            new_insts = []
            for inst in bb.get('instructions', []):
                si = inst.get('sync_info')
                ow = (si or {}).get('on_wait') or []
                eng = inst.get('engine', 'Unassigned')
                if len(ow) > 1 and eng != 'Unassigned':
                    for w in ow[:-1]:
                        ctr += 1
                        new_insts.append({
                            'debug': inst.get('debug', 0),
                            'engine': eng,
                            'ins': [],
                            'outs': [],
                            'name': f"WS-{ctr}-{inst['name']}",
                            'opcode': 'Drain',
                            'sync_info': {'on_update': [], 'on_wait': [w]},
                        })
                    si['on_wait'] = ow[-1:]
                    changed = True
                new_insts.append(inst)
            bb['instructions'] = new_insts
    return json.dumps(d).encode() if changed else bir_bytes


_orig_compile = _b2j.compile_bir_kernel


def _patched_compile(ant_bir_str, *args, **kwargs):
    return _orig_compile(_split_multiwaits(ant_bir_str), *args, **kwargs)


if _b2j.compile_bir_kernel.__name__ != '_patched_compile':
    _b2j.compile_bir_kernel = _patched_compile
# ---------------------------------------------------------------------------


def _bcast(t_ap, col0, pat):
    """AP over tile `t_ap`'s partitions starting at free column col0 with
    custom free [step, count] dims (step 0 = broadcast)."""
    base = t_ap[:, col0:col0 + 1]
    return bass.AP(tensor=base.tensor, offset=base.offset,
                   ap=[base.ap[0]] + [list(p) for p in pat])


def _pstride(t_ap, pstep, pcount, width):
    """AP reading `pcount` partitions with stride `pstep`, `width` cols.
    Partition stride is in flattened elements: pstep * per-partition row len."""
    row = t_ap.ap[0][0]
    return bass.AP(tensor=t_ap.tensor, offset=t_ap.offset,
                   ap=[[pstep * row, pcount], [1, width]])


def _stage_products(nc, eng, out_s, in_a, in_b, op):
    """out_s[:, slot] = in_a[:, a] (op) in_b[:, b] over the SEGS layout."""
    col = 0
    for a0, a1, bcnt in SEGS:
        na = a1 - a0
        n = na * bcnt
        eng.tensor_tensor(
            out=out_s[:, col:col + n].rearrange("p (a b) -> p a b", a=na),
            in0=_bcast(in_a, a0, [[1, na], [0, bcnt]]),
            in1=_bcast(in_b, 0, [[0, na], [1, bcnt]]),
            op=op,
        )
        col += n


def _topk32(nc, work, width, vals, pos, sentinel, widths=None):
    """Top-32 of work[:, :width] per partition -> vals [.,32] f32 descending,
    pos [.,32] u16 positions. Mutates work (found values -> sentinel).
    widths: optional per-round prefix widths (round r's top-8 provably lies
    within the first widths[r] slots; each must cover the previous)."""
    if widths is None:
        widths = [width] * 4
    for r in range(4):
        w = widths[r]
        v8 = vals[:, r * 8:(r + 1) * 8]
        nc.vector.max(out=v8, in_=work[:, :w])
        nc.vector.max_index(out=pos[:, r * 8:(r + 1) * 8], in_max=v8,
                            in_values=work[:, :w])
        if r < 3:
            nc.vector.match_replace(out=work[:, :w], in_to_replace=v8,
                                    in_values=work[:, :w],
                                    imm_value=sentinel)


def _wrap(nc, pool, pos, mdiag, name):
    """Wrapped gather positions: wpos[p, s] = pos[p, s*16 + p%16] via
    per-half multiply-accumulate on gpsimd (u16, sums < 256, exact)."""
    OP = mybir.AluOpType
    scr = pool.tile([NP, K], u16, name=f"wscr{name}")
    wpos = pool.tile([NP, 2], u16, name=f"wpos{name}")
    nc.vector.tensor_tensor(
        out=scr[:, :].rearrange("p (s w) -> p s w", s=2),
        in0=pos[:, :].rearrange("p (s w) -> p s w", s=2),
        in1=_bcast(mdiag, 0, [[0, 2], [1, 16]]),
        op=OP.mult)
    with nc.allow_low_precision(reason="u16 pos sums < 256, exact"):
        nc.vector.tensor_reduce(
            out=wpos[:, :],
            in_=scr[:, :].rearrange("p (s w) -> p s w", s=2),
            axis=mybir.AxisListType.X, op=OP.add)
    return wpos


STOP_AFTER = int(os.environ.get("KSTOP", "99"))


def _build_module():
    nc = bass.Bass()
    z_d = nc.dram_tensor("z", [RPC, U * DP], f32, kind="ExternalInput")
    lt_d = nc.dram_tensor("log_tau", [1, 1], f32, kind="ExternalInput")
    w_d = nc.dram_tensor("w_out", [RPC, K], f32, kind="ExternalOutput")
    i_d = nc.dram_tensor("idx_out", [RPC, K], f32, kind="ExternalOutput")

    AF = mybir.ActivationFunctionType
    OP = mybir.AluOpType

    with TileContext(nc) as tc:
        with tc.tile_pool(name="p", bufs=1) as pool, \
             tc.tile_pool(name="pp", bufs=1, space="PSUM") as ppool:
            # ---- iotas first (unblock DVE-side mdiag setup), then input
            # DMAs (z on sync, lt on gpsimd), then bigger consts ----
            zt = pool.tile([NU, DP], f32)
            z_ap = z_d[:, :]
            # u = f*8 + r -> dram offset f*DP + r*U*DP; split factors {0,1}
            # (sync) / factor 2 (scalar) for lower input latency
            z01 = bass.AP(tensor=z_ap.tensor, offset=z_ap.offset,
                          ap=[[DP, 2], [U * DP, RPC], [1, DP]])
            z2 = bass.AP(tensor=z_ap.tensor, offset=z_ap.offset + 2 * DP,
                         ap=[[U * DP, RPC], [1, DP]])
            nc.sync.dma_start(zt[0:2 * RPC, :], z01)
            nc.scalar.dma_start(zt[2 * RPC:NU, :], z2)
            lt = pool.tile([NU, 1], f32)
            lt_ap = lt_d[:, :]
            lt_src = bass.AP(tensor=lt_ap.tensor, offset=lt_ap.offset,
                             ap=[[0, NU], [1, 1]])
            nc.gpsimd.dma_start(lt[:, :], lt_src)
            pid = pool.tile([NP, 1], u32)
            nc.gpsimd.iota(pid[:, :], pattern=[[0, 1]], base=0,
                           channel_multiplier=1)
            iw = pool.tile([NP, 16], u32)
            nc.gpsimd.iota(iw[:, :], pattern=[[1, 16]], base=0,
                           channel_multiplier=0)

            # wrap-diagonal mdiag[p, w] = (w == p % 16); DVE runs these
            # while waiting for z
            pmod = pool.tile([NP, 1], u32)
            nc.vector.tensor_scalar(out=pmod[:, :], in0=pid[:, :],
                                    scalar1=15, scalar2=None,
                                    op0=OP.bitwise_and)
            pmodf = pool.tile([NP, 1], f32)
            nc.vector.tensor_copy(pmodf[:, :], pmod[:, :])
            iwf = pool.tile([NP, 16], f32)
            nc.vector.tensor_copy(iwf[:, :], iw[:, :])
            mdiag = pool.tile([NP, 16], u16)
            nc.vector.tensor_scalar(out=mdiag[:, :], in0=iwf[:, :],
                                    scalar1=pmodf[:, 0:1], scalar2=None,
                                    op0=OP.is_equal)

            # one-hot replication matrices M[f][u, d] = (u == f*8 + d//16)
            ones24 = pool.tile([NU, NP], f32)
            nc.gpsimd.memset(ones24[:, :], 1.0)
            M = []
            Mi = []
            for f in range(U):
                mf = pool.tile([NU, NP], f32, name=f"M{f}")
                nc.gpsimd.affine_select(
                    out=mf[:, :].rearrange("p (a b) -> p a b", a=RPC),
                    in_=ones24[:, :].rearrange("p (a b) -> p a b", a=RPC),
                    pattern=[[-1, RPC], [0, 16]],
                    base=-8 * f, channel_multiplier=1,
                    compare_op=OP.is_equal, fill=0.0)
                M.append(mf)
            for f, sc in ((0, 16384.0), (1, 128.0)):
                mfi = pool.tile([NU, NP], f32, name=f"Mi{f}")
                nc.gpsimd.tensor_scalar(out=mfi[:, :], in0=M[f][:, :],
                                        scalar1=sc, scalar2=None,
                                        op0=OP.mult)
                Mi.append(mfi)
            Mi.append(M[2])

            # ---- softmax, bit-matching the reference lowering ----
            rtau = pool.tile([NU, 1], f32)
            nc.scalar.activation(out=rtau[:, :], in_=lt[:, :], func=AF.Exp,
                                 bias=0.0, scale=-1.0)
            nm = pool.tile([NU, 1], f32)
            nc.vector.tensor_reduce(out=nm[:, :], in_=zt[:, :],
                                    axis=mybir.AxisListType.X,
                                    op=OP.max, negate=True)
            # rhs for replication matmuls: [vals | rowsum-recip | fidx*128]
            rhs = pool.tile([NU, 2 * K + 1], f32)
            et = pool.tile([NU, DP], f32)
            s = pool.tile([NU, 1], f32)
            nc.scalar.activation(out=et[:, :], in_=zt[:, :], func=AF.Exp,
                                 bias=nm[:, 0:1], scale=rtau[:, 0:1],
                                 accum_out=s[:, :])

            if STOP_AFTER == 1:
                nc.gpsimd.dma_start(w_d[:, :], _pstride(et, 1, RPC, K))
                nc.gpsimd.dma_start(i_d[:, :], _pstride(rhs, 1, RPC, K))
                return nc

            # ---- joint factor top-32 on raw exp values (normalization is
            # a uniform positive per-unit scale: same order, verified
            # tie-free), then normalize just the 32 selected values ----
            fidx = pool.tile([NU, K], u16)
            for r in range(4):
                v8 = rhs[:, r * 8:(r + 1) * 8]
                nc.vector.max(out=v8, in_=et[:, :])
                if r == 0:
                    nc.vector.reciprocal(rhs[:, K:K + 1], s[:, :])
                nc.vector.max_index(out=fidx[:, r * 8:(r + 1) * 8],
                                    in_max=v8, in_values=et[:, :])
                if r < 3:
                    nc.vector.match_replace(out=et[:, :], in_to_replace=v8,
                                            in_values=et[:, :],
                                            imm_value=-1.0)
            nc.vector.tensor_copy(rhs[:, K + 1:2 * K + 1], fidx[:, :])

            # ---- replicate to 16-partition row groups via PE, vals first
            # so DVE stage-A products can start ASAP ----
            pv, pf = [], []
            for f in range(U):
                t = ppool.tile([NP, K + 1], f32, name=f"pv{f}")
                nc.tensor.matmul(t[:, :], M[f][:, :], rhs[:, 0:K + 1],
                                 start=True, stop=True)
                pv.append(t)
            for f in range(U):
                t = ppool.tile([NP, K], f32, name=f"pf{f}")
                nc.tensor.matmul(t[:, :], Mi[f][:, :],
                                 rhs[:, K + 1:2 * K + 1],
                                 start=True, stop=True)
                pf.append(t)
            # normalize replicated vals into SBUF (scalar input from PSUM is
            # exempt from the one-PSUM-input rule); fidx halves via ACT
            vn = []
            for f in range(U):
                t = pool.tile([NP, K], f32, name=f"vn{f}")
                nc.vector.tensor_scalar(out=t[:, :], in0=pv[f][:, 0:K],
                                        scalar1=pv[f][:, K:K + 1],
                                        scalar2=None, op0=OP.mult)
                vn.append(t)
            v0sb = vn[0]
            fsb = []
            for f in range(U):
                ff = pool.tile([NP, K], f32, name=f"fsb{f}")
                nc.scalar.activation(out=ff[:, :], in_=pf[f][:, :],
                                     func=AF.Copy, bias=0.0, scale=1.0)
                fsb.append(ff)

            if STOP_AFTER == 2:
                nc.gpsimd.dma_start(w_d[:, :], _pstride(v0sb, 16, RPC, K))
                nc.gpsimd.dma_start(i_d[:, :], _pstride(fsb[0], 16, RPC, K))
                return nc

            # ---- stage A: pair products (DVE) + pair flat idx (gpsimd) ----
            sA = pool.tile([NP, NSLOT], f32)
            _stage_products(nc, nc.vector, sA, vn[0], vn[1], OP.mult)
            nA = pool.tile([NP, NSLOT], f32)
            _stage_products(nc, nc.gpsimd, nA, fsb[0], fsb[1], OP.add)

            vA = pool.tile([NP, K], f32)
            posA = pool.tile([NP, K], u16)
            _topk32(nc, sA, NSLOT, vA, posA, -1.0, widths=RW)

            if STOP_AFTER == 3:
                nc.gpsimd.dma_start(w_d[:, :], _pstride(vA, 16, RPC, K))
                nc.gpsimd.dma_start(i_d[:, :], _pstride(nA, 16, RPC, K))
                return nc

            # ---- gather pair flat indices at posA (gpsimd) ----
            wposA = _wrap(nc, pool, posA, mdiag, "A")
            gA = pool.tile([NP, K], f32)
            nc.gpsimd.indirect_copy(out=gA[:, :], data=nA[:, :],
                                    idxs=wposA[:, :],
                                    i_know_ap_gather_is_preferred=True)

            # ---- stage B: triple products + flat indices ----
            sB = pool.tile([NP, NSLOT], f32)
            _stage_products(nc, nc.vector, sB, vA, vn[2], OP.mult)
            nB = pool.tile([NP, NSLOT], f32)
            _stage_products(nc, nc.gpsimd, nB, gA, fsb[2], OP.add)

            vB = pool.tile([NP, K], f32)
            posB = pool.tile([NP, K], u16)
            _topk32(nc, sB, NSLOT, vB, posB, -1.0, widths=RW)

            if STOP_AFTER == 4:
                nc.gpsimd.dma_start(w_d[:, :], _pstride(vB, 16, RPC, K))
                nc.gpsimd.dma_start(i_d[:, :], _pstride(nB, 16, RPC, K))
                return nc

            nc.sync.dma_start(w_d[:, :], _pstride(vB, 16, RPC, K))
            wposB = _wrap(nc, pool, posB, mdiag, "B")
            gB = pool.tile([NP, K], f32)
            nc.gpsimd.indirect_copy(out=gB[:, :], data=nB[:, :],
                                    idxs=wposB[:, :],
                                    i_know_ap_gather_is_preferred=True)
            nc.sync.dma_start(i_d[:, :], _pstride(gB, 16, RPC, K))
    return nc


LAST_RESULTS = None


def kernel(z, log_tau, _trace=False):
    z = np.ascontiguousarray(np.asarray(z, dtype=np.float32))
    log_tau = np.asarray(log_tau, dtype=np.float32).reshape(1, 1)
    assert z.shape == (B, U * DP), z.shape

    nc = _build_module()
    in_maps = []
    for c in range(NCORES):
        in_maps.append({
            "z": z[c * RPC:(c + 1) * RPC],
            "log_tau": log_tau,
        })
    global LAST_RESULTS
    kw = {}
    if _trace:
        kw = dict(trace=True, trace_cores=[0])
    res = run_bass_kernel_spmd(nc, in_maps, core_ids=list(range(NCORES)), **kw)
    LAST_RESULTS = res
    indices = np.concatenate([r["idx_out"] for r in res.results], axis=0)
    weights = np.concatenate([r["w_out"] for r in res.results], axis=0)
    return indices.astype(np.int32), weights.astype(np.float32)


if __name__ == "__main__":
    z = np.load('/tmp/z.npy')
    lt = np.load('/tmp/logtau.npy')
    ind, w = kernel(z, lt)
    print(ind[:2], w[:2])

